# revision 5
# baseline (speedup 1.0000x reference)
"""Attention-LSTM (CaptioningRNN) Trainium2 kernel.

Strategy: data-parallel over the batch N=128 across 8 NeuronCores (16
samples/core), zero cross-core communication.  Per core:

  Phase 1:  xW = x_local @ Wx + b  (f32r matmuls, full 128-row PE tiles)
            -> bf16 DRAM scratch, rows ordered (t, n);
            c0 = h0 = mean_m(A_local)   (DVE reduce).
  Phase 2:  64 sequential LSTM steps.  The two recurrent GEMMs
            (h @ Wh, attn @ Wattn) run as bf16 matmuls: transposed state
            chunks are the stationary operand, SBUF-resident bf16 weights
            are the moving operand; xW_t is folded into the same PSUM
            accumulation via an identity matmul.  Attention:
              scores  = hT.T @ A_chunks (bf16 PE) -> diagonal extraction
                        via mask-multiply + reduce (DVE);
              softmax = ACT exp + DVE reduce/reciprocal;
              attn^T  = softmax weights broadcast across partitions with
                        a ones-matmul, then DVE multiply + reduce,
                        produced directly in the transposed layout the
                        next GEMM needs.
"""

import sys

sys.path.insert(0, "/opt/trn_rl_repo")

import ml_dtypes
import numpy as np

import concourse.bass as bass  # noqa: F401
import concourse.mybir as mybir
import concourse.tile as tile
from concourse import bacc
from concourse.bass_utils import run_bass_kernel_spmd

F32 = mybir.dt.float32
F32R = mybir.dt.float32r
BF16 = mybir.dt.bfloat16

N, T, D, H = 128, 64, 1024, 1024
K4 = 4 * H            # 4096
NCORES = 8
NL = N // NCORES      # 16 samples per core
M = 16                # spatial positions (4x4)
HC = H // 128         # 8 h-chunks
SCALE = 1.0 / float(np.sqrt(H))

_cache = {}


def _build():
    if "nc" in _cache:
        return _cache["nc"]

    nc = bacc.Bacc("TRN2", target_bir_lowering=False)

    # ---- kernel I/O ----------------------------------------------------
    # xT rows are D, columns are (t, n) so step t's block is contiguous.
    d_xT = nc.dram_tensor("xT", [D, T * NL], BF16, kind="ExternalInput")
    d_A = nc.dram_tensor("A", [NL, H, M], F32, kind="ExternalInput")
    d_Wx = nc.dram_tensor("Wx", [D, K4], BF16, kind="ExternalInput")
    d_Wh = nc.dram_tensor("Wh", [H, K4], BF16, kind="ExternalInput")
    d_Wa = nc.dram_tensor("Wa", [H, K4], BF16, kind="ExternalInput")
    d_b = nc.dram_tensor("b", [1, K4], BF16, kind="ExternalInput")
    d_id = nc.dram_tensor("ident", [16, 16], F32, kind="ExternalInput")
    d_mmn = nc.dram_tensor("mask_mn", [16, 16 * 16], F32, kind="ExternalInput")
    d_mnm = nc.dram_tensor("mask_nm", [16, 16 * 16], F32, kind="ExternalInput")
    d_ones = nc.dram_tensor("ones16", [16, 128], BF16, kind="ExternalInput")
    d_y = nc.dram_tensor("y", [NL, T, H], F32, kind="ExternalOutput")

    d_xw = nc.dram_tensor("xw_scratch", [T * NL, K4], BF16)

    with tile.TileContext(nc) as tc:
      with tc.tile_pool(name="state", bufs=1) as stp:
        c_sb = stp.tile([NL, H], F32, tag="c")

        # ============== Phase 1: xW = x @ Wx + b, c0 ====================
        with (
            tc.tile_pool(name="p1w", bufs=1) as p1w,
            tc.tile_pool(name="p1s", bufs=2) as p1s,
            tc.tile_pool(name="p1p", bufs=4, space="PSUM") as p1p,
        ):
            wx = p1w.tile([128, HC, K4], BF16, tag="wx")
            nc.sync.dma_start(
                out=wx[:], in_=d_Wx.rearrange("(kc p) f -> p kc f", p=128)
            )
            bias = p1w.tile([1, K4], BF16, tag="bias")
            nc.sync.dma_start(out=bias[:], in_=d_b[:])
            ones1 = p1w.tile([1, 128], BF16, tag="ones1")
            nc.vector.memset(ones1[:], 1.0)

            xt = p1w.tile([128, HC, T * NL], BF16, tag="xt")
            nc.sync.dma_start(
                out=xt[:], in_=d_xT.rearrange("(kc p) r -> p kc r", p=128)
            )

            # c0 = mean over m of A  (layout [n, h])
            for hh in range(4):
                hs = H // 4
                a_n = p1s.tile([NL, hs, M], F32, tag="a_n")
                nc.sync.dma_start(
                    out=a_n[:], in_=d_A[:, hh * hs : (hh + 1) * hs, :]
                )
                csum = p1s.tile([NL, hs], F32, tag="csum")
                nc.vector.tensor_reduce(
                    csum[:], a_n[:], axis=mybir.AxisListType.X, op=mybir.AluOpType.add
                )
                nc.scalar.mul(c_sb[:, hh * hs : (hh + 1) * hs], csum[:], 1.0 / M)

            # xW GEMM over 8 row-chunks of (t, n)
            for mc in range(HC):
                for j in range(8):
                    pj = p1p.tile([128, 512], F32, tag="p1psum")
                    for kc in range(HC):
                        nc.tensor.matmul(
                            pj[:],
                            xt[:, kc, mc * 128 : (mc + 1) * 128],
                            wx[:, kc, j * 512 : (j + 1) * 512],
                            start=(kc == 0),
                            stop=False,
                        )
                    nc.tensor.matmul(
                        pj[:],
                        ones1[:],
                        bias[:, j * 512 : (j + 1) * 512],
                        start=False,
                        stop=True,
                    )
                    ob = p1s.tile([128, 512], BF16, tag="p1out")
                    nc.vector.tensor_copy(ob[:], pj[:])
                    nc.sync.dma_start(
                        out=d_xw[mc * 128 : (mc + 1) * 128, j * 512 : (j + 1) * 512],
                        in_=ob[:],
                    )

        # ============== Phase 2: recurrent loop =========================
        with (
            tc.tile_pool(name="wts", bufs=1) as wts,
            tc.tile_pool(name="stt", bufs=1) as stt,
            tc.tile_pool(name="xwp", bufs=2) as xwp,
            tc.tile_pool(name="gat", bufs=1) as gat,
            tc.tile_pool(name="sml", bufs=2) as sml,
            tc.tile_pool(name="big", bufs=1) as big,
            tc.tile_pool(name="actp", bufs=3, space="PSUM") as actp,
            tc.tile_pool(name="trp", bufs=2, space="PSUM") as trp,
            tc.tile_pool(name="scp", bufs=1, space="PSUM") as scp,
        ):
            wh = wts.tile([128, HC, K4], BF16, tag="wh")
            nc.sync.dma_start(out=wh[:], in_=d_Wh.rearrange("(kc p) f -> p kc f", p=128))
            wa = wts.tile([128, HC, K4], BF16, tag="wa")
            nc.sync.dma_start(out=wa[:], in_=d_Wa.rearrange("(kc p) f -> p kc f", p=128))
            # A[n, hc*128+p, m] in transposed per-chunk layout, bf16
            at4 = wts.tile([128, HC, NL, M], BF16, tag="at4")
            for hc in range(HC):
                a_stage = sml.tile([128, NL, M], F32, tag="a_stage")
                nc.sync.dma_start(
                    out=a_stage[:],
                    in_=d_A.rearrange("n (hc p) m -> hc p n m", hc=HC)[hc],
                )
                nc.vector.tensor_copy(at4[:, hc], a_stage[:])
            id16 = wts.tile([16, 16], F32, tag="id16")
            nc.sync.dma_start(out=id16[:], in_=d_id[:])
            mask_mn = wts.tile([16, M, NL], F32, tag="mask_mn")
            nc.sync.dma_start(
                out=mask_mn[:], in_=d_mmn.rearrange("p (a c) -> p a c", a=M)
            )
            mask_nm = wts.tile([16, NL, M], F32, tag="mask_nm")
            nc.sync.dma_start(
                out=mask_nm[:], in_=d_mnm.rearrange("p (a c) -> p a c", a=NL)
            )
            ones16 = wts.tile([16, 128], BF16, tag="ones16")
            nc.sync.dma_start(out=ones16[:], in_=d_ones[:])
            id16b = wts.tile([16, 16], BF16, tag="id16b")
            nc.vector.tensor_copy(id16b[:], id16[:])

            h_sb = stt.tile([NL, H], F32, tag="h")
            nc.vector.tensor_copy(h_sb[:], c_sb[:])

            hT_b = stt.tile([128, HC, NL], BF16, tag="hT_b")
            aT_b = stt.tile([128, HC, NL], BF16, tag="aT_b")

            def transposes(src_sb):
                for k in range(HC):
                    pt = trp.tile([128, NL], F32, tag="trps")
                    nc.tensor.transpose(
                        pt[:], src_sb[:, k * 128 : (k + 1) * 128], id16[:]
                    )
                    nc.vector.tensor_copy(hT_b[:, k], pt[:])

            def attention():
                # scores: S[n, (m, n')] accumulated over h-chunks (bf16 PE)
                ps = scp.tile([16, M * NL], F32, tag="sc_ps")
                for k in range(HC):
                    nc.tensor.matmul(
                        ps[:],
                        hT_b[:, k],
                        at4[:, k].rearrange("p n m -> p m n"),
                        start=(k == 0),
                        stop=(k == HC - 1),
                    )
                smul = sml.tile([16, M, NL], F32, tag="smul")
                nc.vector.tensor_mul(
                    smul[:], ps[:].rearrange("p (m n) -> p m n", m=M), mask_mn[:]
                )
                sc = sml.tile([16, M], F32, tag="sc")
                nc.vector.tensor_reduce(
                    sc[:], smul[:], axis=mybir.AxisListType.X, op=mybir.AluOpType.add
                )
                # softmax (1/sqrt(H) scale folded into exp)
                mx = sml.tile([16, 1], F32, tag="mx")
                nc.vector.tensor_reduce(
                    mx[:], sc[:], axis=mybir.AxisListType.X, op=mybir.AluOpType.max
                )
                nb = sml.tile([16, 1], F32, tag="nb")
                nc.scalar.mul(nb[:], mx[:], -SCALE)
                ex = sml.tile([16, M], F32, tag="ex")
                nc.scalar.activation(
                    ex[:], sc[:], mybir.ActivationFunctionType.Exp,
                    bias=nb[:], scale=SCALE,
                )
                sm = sml.tile([16, 1], F32, tag="sm")
                nc.vector.tensor_reduce(
                    sm[:], ex[:], axis=mybir.AxisListType.X, op=mybir.AluOpType.add
                )
                rc = sml.tile([16, 1], F32, tag="rc")
                nc.vector.reciprocal(rc[:], sm[:])
                w16 = sml.tile([16, M], F32, tag="w16")
                nc.vector.tensor_scalar_mul(w16[:], ex[:], rc[:])
                # wB[p, (n, m)] = w[n, m] on every partition p
                wd = sml.tile([16, NL, M], BF16, tag="wd")
                nc.vector.tensor_mul(
                    wd[:],
                    w16[:].unsqueeze(1).broadcast_to([16, NL, M]),
                    mask_nm[:],
                )
                pwb = scp.tile([128, NL * M], F32, tag="wb_ps")
                nc.tensor.matmul(
                    pwb[:],
                    ones16[:],
                    wd[:].rearrange("p n m -> p (n m)"),
                    start=True,
                    stop=True,
                )
                wbs = sml.tile([128, NL, M], BF16, tag="wbs")
                nc.vector.tensor_copy(
                    wbs[:], pwb[:].rearrange("p (n m) -> p n m", n=NL)
                )
                # attnT[p, (hc, n)] = sum_m A[n, hc*128+p, m] * w[n, m]
                tmp = big.tile([128, HC, NL, M], BF16, tag="attmp")
                nc.vector.tensor_mul(
                    tmp[:],
                    at4[:],
                    wbs[:].unsqueeze(1).broadcast_to([128, HC, NL, M]),
                )
                atf = sml.tile([128, HC, NL], F32, tag="atf")
                nc.vector.tensor_reduce(
                    atf[:], tmp[:], axis=mybir.AxisListType.X, op=mybir.AluOpType.add
                )
                nc.vector.tensor_copy(aT_b[:], atf[:])

            # initial state: h = c0; hT and attn for step 0
            transposes(h_sb)
            attention()

            for t in range(T):
                gi = gat.tile([NL, H], F32, tag="gi")
                gf = gat.tile([NL, H], F32, tag="gf")
                go = gat.tile([NL, H], F32, tag="go")
                gg = gat.tile([NL, H], F32, tag="gg")
                gates = [gi, gf, go, gg]
                for j in range(8):
                    xwt = xwp.tile([NL, 512], BF16, tag="xwt")
                    nc.sync.dma_start(
                        out=xwt[:],
                        in_=d_xw[t * NL : (t + 1) * NL, j * 512 : (j + 1) * 512],
                    )
                    pj = actp.tile([NL, 512], F32, tag="act_ps")
                    for k in range(HC):
                        nc.tensor.matmul(
                            pj[:], hT_b[:, k], wh[:, k, j * 512 : (j + 1) * 512],
                            start=(k == 0), stop=False,
                        )
                    for k in range(HC):
                        nc.tensor.matmul(
                            pj[:], aT_b[:, k], wa[:, k, j * 512 : (j + 1) * 512],
                            start=False, stop=False,
                        )
                    nc.tensor.matmul(
                        pj[:], id16b[:], xwt[:],
                        start=False, stop=True,
                    )
                    g = j // 2
                    half = (j % 2) * 512
                    func = (
                        mybir.ActivationFunctionType.Tanh
                        if g == 3
                        else mybir.ActivationFunctionType.Sigmoid
                    )
                    nc.scalar.activation(gates[g][:, half : half + 512], pj[:], func)

                # c = f*c + i*g ; h = o * tanh(c)
                fc = gat.tile([NL, H], F32, tag="fc")
                nc.vector.tensor_mul(fc[:], gf[:], c_sb[:])
                ig = gat.tile([NL, H], F32, tag="ig")
                nc.vector.tensor_mul(ig[:], gi[:], gg[:])
                nc.vector.tensor_add(c_sb[:], fc[:], ig[:])
                th = gat.tile([NL, H], F32, tag="th")
                nc.scalar.activation(th[:], c_sb[:], mybir.ActivationFunctionType.Tanh)
                nc.vector.tensor_mul(h_sb[:], go[:], th[:])

                nc.sync.dma_start(out=d_y[:, t, :], in_=h_sb[:])

                if t < T - 1:
                    transposes(h_sb)
                    attention()

    nc.compile()
    _cache["nc"] = nc
    return nc


def kernel(x, A, Wx, Wh, Wattn, b):
    x = np.ascontiguousarray(np.asarray(x, dtype=np.float32))
    A = np.ascontiguousarray(np.asarray(A, dtype=np.float32))
    Wxb = np.ascontiguousarray(np.asarray(Wx, dtype=np.float32).astype(ml_dtypes.bfloat16))
    Whb = np.ascontiguousarray(np.asarray(Wh, dtype=np.float32).astype(ml_dtypes.bfloat16))
    Wab = np.ascontiguousarray(np.asarray(Wattn, dtype=np.float32).astype(ml_dtypes.bfloat16))
    b2 = np.ascontiguousarray(
        np.asarray(b, dtype=np.float32).reshape(1, K4).astype(ml_dtypes.bfloat16)
    )

    ident = np.eye(16, dtype=np.float32)
    mask_mn = np.zeros((16, M * NL), dtype=np.float32)
    mask_nm = np.zeros((16, NL * M), dtype=np.float32)
    for a in range(M):
        for n in range(NL):
            mask_mn[n, a * NL + n] = 1.0  # [n, (m, n')]
            mask_nm[n, n * M + a] = 1.0   # [n', (n, m)]
    ones16 = np.ones((16, 128), dtype=ml_dtypes.bfloat16)

    nc = _build()
    in_maps = []
    for k in range(NCORES):
        xs = x[k * NL : (k + 1) * NL]                     # [16, 64, 1024]
        xT = np.ascontiguousarray(
            xs.transpose(1, 0, 2).reshape(T * NL, D).T.astype(ml_dtypes.bfloat16)
        )
        Ak = np.ascontiguousarray(A[k * NL : (k + 1) * NL].reshape(NL, H, M))
        in_maps.append(
            {
                "xT": xT,
                "A": Ak,
                "Wx": Wxb,
                "Wh": Whb,
                "Wa": Wab,
                "b": b2,
                "ident": ident,
                "mask_mn": mask_mn,
                "mask_nm": mask_nm,
                "ones16": ones16,
            }
        )

    _cache["in_maps"] = in_maps
    res = run_bass_kernel_spmd(nc, in_maps, core_ids=list(range(NCORES)))
    out = np.concatenate([res.results[k]["y"] for k in range(NCORES)], axis=0)
    return out.astype(np.float32)


# revision 7
# speedup vs baseline: 1.0230x; 1.0230x over previous
"""Attention-LSTM (CaptioningRNN) Trainium2 kernel.

Strategy: data-parallel over the batch N=128 across 8 NeuronCores (16
samples/core), zero cross-core communication.  Per core:

  Phase 1:  xW = x_local @ Wx + b  (f32r matmuls, full 128-row PE tiles)
            -> bf16 DRAM scratch, rows ordered (t, n);
            c0 = h0 = mean_m(A_local)   (DVE reduce).
  Phase 2:  64 sequential LSTM steps.  The two recurrent GEMMs
            (h @ Wh, attn @ Wattn) run as bf16 matmuls: transposed state
            chunks are the stationary operand, SBUF-resident bf16 weights
            are the moving operand; xW_t is folded into the same PSUM
            accumulation via an identity matmul.  Attention:
              scores  = hT.T @ A_chunks (bf16 PE) -> diagonal extraction
                        via mask-multiply + reduce (DVE);
              softmax = ACT exp + DVE reduce/reciprocal;
              attn^T  = softmax weights broadcast across partitions with
                        a ones-matmul, then DVE multiply + reduce,
                        produced directly in the transposed layout the
                        next GEMM needs.
"""

import sys

sys.path.insert(0, "/opt/trn_rl_repo")

import ml_dtypes
import numpy as np

import concourse.bass as bass  # noqa: F401
import concourse.mybir as mybir
import concourse.tile as tile
from concourse import bacc
from concourse.bass_utils import run_bass_kernel_spmd

F32 = mybir.dt.float32
F32R = mybir.dt.float32r
BF16 = mybir.dt.float16  # IEEE fp16: same PE rate as bf16, 4x the mantissa precision

N, T, D, H = 128, 64, 1024, 1024
K4 = 4 * H            # 4096
NCORES = 8
NL = N // NCORES      # 16 samples per core
M = 16                # spatial positions (4x4)
HC = H // 128         # 8 h-chunks
SCALE = 1.0 / float(np.sqrt(H))

_cache = {}


def _build():
    if "nc" in _cache:
        return _cache["nc"]

    nc = bacc.Bacc("TRN2", target_bir_lowering=False)

    # ---- kernel I/O ----------------------------------------------------
    # xT rows are D, columns are (t, n) so step t's block is contiguous.
    d_xT = nc.dram_tensor("xT", [D, T * NL], BF16, kind="ExternalInput")
    d_A = nc.dram_tensor("A", [NL, H, M], F32, kind="ExternalInput")
    d_Wx = nc.dram_tensor("Wx", [D, K4], BF16, kind="ExternalInput")
    d_Wh = nc.dram_tensor("Wh", [H, K4], BF16, kind="ExternalInput")
    d_Wa = nc.dram_tensor("Wa", [H, K4], BF16, kind="ExternalInput")
    d_b = nc.dram_tensor("b", [1, K4], BF16, kind="ExternalInput")
    d_id = nc.dram_tensor("ident", [16, 16], F32, kind="ExternalInput")
    d_mmn = nc.dram_tensor("mask_mn", [16, 16 * 16], F32, kind="ExternalInput")
    d_mnm = nc.dram_tensor("mask_nm", [16, 16 * 16], F32, kind="ExternalInput")
    d_ones = nc.dram_tensor("ones16", [16, 128], BF16, kind="ExternalInput")
    d_ssum = nc.dram_tensor("ssum", [128, 16], BF16, kind="ExternalInput")
    d_y = nc.dram_tensor("y", [NL, T, H], F32, kind="ExternalOutput")

    d_xw = nc.dram_tensor("xw_scratch", [T * NL, K4], BF16)

    with tile.TileContext(nc) as tc:
      with tc.tile_pool(name="state", bufs=1) as stp:
        c_sb = stp.tile([NL, H], F32, tag="c")

        # ============== Phase 1: xW = x @ Wx + b, c0 ====================
        with (
            tc.tile_pool(name="p1w", bufs=1) as p1w,
            tc.tile_pool(name="p1s", bufs=2) as p1s,
            tc.tile_pool(name="p1p", bufs=4, space="PSUM") as p1p,
        ):
            wx = p1w.tile([128, HC, K4], BF16, tag="wx")
            nc.sync.dma_start(
                out=wx[:], in_=d_Wx.rearrange("(kc p) f -> p kc f", p=128)
            )
            bias = p1w.tile([1, K4], BF16, tag="bias")
            nc.sync.dma_start(out=bias[:], in_=d_b[:])
            ones1 = p1w.tile([1, 128], BF16, tag="ones1")
            nc.vector.memset(ones1[:], 1.0)

            xt = p1w.tile([128, HC, T * NL], BF16, tag="xt")
            nc.sync.dma_start(
                out=xt[:], in_=d_xT.rearrange("(kc p) r -> p kc r", p=128)
            )

            # c0 = mean over m of A  (layout [n, h])
            for hh in range(4):
                hs = H // 4
                a_n = p1s.tile([NL, hs, M], F32, tag="a_n")
                nc.sync.dma_start(
                    out=a_n[:], in_=d_A[:, hh * hs : (hh + 1) * hs, :]
                )
                csum = p1s.tile([NL, hs], F32, tag="csum")
                nc.vector.tensor_reduce(
                    csum[:], a_n[:], axis=mybir.AxisListType.X, op=mybir.AluOpType.add
                )
                nc.scalar.mul(c_sb[:, hh * hs : (hh + 1) * hs], csum[:], 1.0 / M)

            # xW GEMM over 8 row-chunks of (t, n)
            for mc in range(HC):
                for j in range(8):
                    pj = p1p.tile([128, 512], F32, tag="p1psum")
                    for kc in range(HC):
                        nc.tensor.matmul(
                            pj[:],
                            xt[:, kc, mc * 128 : (mc + 1) * 128],
                            wx[:, kc, j * 512 : (j + 1) * 512],
                            start=(kc == 0),
                            stop=False,
                        )
                    nc.tensor.matmul(
                        pj[:],
                        ones1[:],
                        bias[:, j * 512 : (j + 1) * 512],
                        start=False,
                        stop=True,
                    )
                    ob = p1s.tile([128, 512], BF16, tag="p1out")
                    nc.vector.tensor_copy(ob[:], pj[:])
                    nc.sync.dma_start(
                        out=d_xw[mc * 128 : (mc + 1) * 128, j * 512 : (j + 1) * 512],
                        in_=ob[:],
                    )

        # ============== Phase 2: recurrent loop =========================
        with (
            tc.tile_pool(name="wts", bufs=1) as wts,
            tc.tile_pool(name="stt", bufs=1) as stt,
            tc.tile_pool(name="xwp", bufs=2) as xwp,
            tc.tile_pool(name="gat", bufs=1) as gat,
            tc.tile_pool(name="sml", bufs=2) as sml,
            tc.tile_pool(name="big", bufs=1) as big,
            tc.tile_pool(name="actp", bufs=2, space="PSUM") as actp,
            tc.tile_pool(name="packp", bufs=2, space="PSUM") as packp,
            tc.tile_pool(name="trp", bufs=2, space="PSUM") as trp,
            tc.tile_pool(name="scp", bufs=1, space="PSUM") as scp,
        ):
            wh = wts.tile([128, HC, K4], BF16, tag="wh")
            nc.sync.dma_start(out=wh[:], in_=d_Wh.rearrange("(kc p) f -> p kc f", p=128))
            wa = wts.tile([128, HC, K4], BF16, tag="wa")
            nc.sync.dma_start(out=wa[:], in_=d_Wa.rearrange("(kc p) f -> p kc f", p=128))
            # A[n, hc*128+p, m] in transposed per-chunk layout, bf16
            at4 = wts.tile([128, HC, NL, M], BF16, tag="at4")
            for hc in range(HC):
                a_stage = sml.tile([128, NL, M], F32, tag="a_stage")
                nc.sync.dma_start(
                    out=a_stage[:],
                    in_=d_A.rearrange("n (hc p) m -> hc p n m", hc=HC)[hc],
                )
                nc.vector.tensor_copy(at4[:, hc], a_stage[:])
            id16 = wts.tile([16, 16], F32, tag="id16")
            nc.sync.dma_start(out=id16[:], in_=d_id[:])
            mask_mn = wts.tile([16, M, NL], F32, tag="mask_mn")
            nc.sync.dma_start(
                out=mask_mn[:], in_=d_mmn.rearrange("p (a c) -> p a c", a=M)
            )
            mask_nm = wts.tile([16, NL, M], F32, tag="mask_nm")
            nc.sync.dma_start(
                out=mask_nm[:], in_=d_mnm.rearrange("p (a c) -> p a c", a=NL)
            )
            ones16 = wts.tile([16, 128], BF16, tag="ones16")
            nc.sync.dma_start(out=ones16[:], in_=d_ones[:])
            id16b = wts.tile([16, 16], BF16, tag="id16b")
            nc.vector.tensor_copy(id16b[:], id16[:])
            ssum = wts.tile([128, 16], BF16, tag="ssum")
            nc.sync.dma_start(out=ssum[:], in_=d_ssum[:])

            h_sb = stt.tile([NL, H], F32, tag="h")
            nc.vector.tensor_copy(h_sb[:], c_sb[:])

            hT_b = stt.tile([128, HC, NL], BF16, tag="hT_b")
            aT_b = stt.tile([128, HC, NL], BF16, tag="aT_b")

            def transposes(src_sb):
                for k in range(HC):
                    pt = trp.tile([128, NL], F32, tag="trps")
                    nc.tensor.transpose(
                        pt[:], src_sb[:, k * 128 : (k + 1) * 128], id16[:]
                    )
                    nc.vector.tensor_copy(hT_b[:, k], pt[:])

            def attention():
                # scores: S[n, (m, n')] accumulated over h-chunks (bf16 PE)
                ps = scp.tile([16, M * NL], F32, tag="sc_ps")
                for k in range(HC):
                    nc.tensor.matmul(
                        ps[:],
                        hT_b[:, k],
                        at4[:, k].rearrange("p n m -> p m n"),
                        start=(k == 0),
                        stop=(k == HC - 1),
                    )
                smul = sml.tile([16, M, NL], F32, tag="smul")
                nc.vector.tensor_mul(
                    smul[:], ps[:].rearrange("p (m n) -> p m n", m=M), mask_mn[:]
                )
                sc = sml.tile([16, M], F32, tag="sc")
                nc.vector.tensor_reduce(
                    sc[:], smul[:], axis=mybir.AxisListType.X, op=mybir.AluOpType.add
                )
                # softmax (1/sqrt(H) scale folded into exp)
                mx = sml.tile([16, 1], F32, tag="mx")
                nc.vector.tensor_reduce(
                    mx[:], sc[:], axis=mybir.AxisListType.X, op=mybir.AluOpType.max
                )
                nb = sml.tile([16, 1], F32, tag="nb")
                nc.scalar.mul(nb[:], mx[:], -SCALE)
                ex = sml.tile([16, M], F32, tag="ex")
                nc.scalar.activation(
                    ex[:], sc[:], mybir.ActivationFunctionType.Exp,
                    bias=nb[:], scale=SCALE,
                )
                sm = sml.tile([16, 1], F32, tag="sm")
                nc.vector.tensor_reduce(
                    sm[:], ex[:], axis=mybir.AxisListType.X, op=mybir.AluOpType.add
                )
                rc = sml.tile([16, 1], F32, tag="rc")
                nc.vector.reciprocal(rc[:], sm[:])
                w16 = sml.tile([16, M], F32, tag="w16")
                nc.vector.tensor_scalar_mul(w16[:], ex[:], rc[:])
                # wB[p, (n, m)] = w[n, m] on every partition p
                wd = sml.tile([16, NL, M], BF16, tag="wd")
                nc.vector.tensor_mul(
                    wd[:],
                    w16[:].unsqueeze(1).broadcast_to([16, NL, M]),
                    mask_nm[:],
                )
                pwb = scp.tile([128, NL * M], F32, tag="wb_ps")
                nc.tensor.matmul(
                    pwb[:],
                    ones16[:],
                    wd[:].rearrange("p n m -> p (n m)"),
                    start=True,
                    stop=True,
                )
                wbs = sml.tile([128, NL, M], BF16, tag="wbs")
                nc.vector.tensor_copy(
                    wbs[:], pwb[:].rearrange("p (n m) -> p n m", n=NL)
                )
                # attnT[p, (hc, n)] = sum_m A[n, hc*128+p, m] * w[n, m]
                tmp = big.tile([128, HC, NL, M], BF16, tag="attmp")
                nc.vector.tensor_mul(
                    tmp[:],
                    at4[:],
                    wbs[:].unsqueeze(1).broadcast_to([128, HC, NL, M]),
                )
                atf = sml.tile([128, HC, NL], F32, tag="atf")
                nc.vector.tensor_reduce(
                    atf[:], tmp[:], axis=mybir.AxisListType.X, op=mybir.AluOpType.add
                )
                nc.vector.tensor_copy(aT_b[:], atf[:])

            # initial state: h = c0; hT and attn for step 0
            transposes(h_sb)
            attention()

            for t in range(T):
                gi = gat.tile([NL, H], F32, tag="gi")
                gf = gat.tile([NL, H], F32, tag="gf")
                go = gat.tile([NL, H], F32, tag="go")
                gg = gat.tile([NL, H], F32, tag="gg")
                gates = [gi, gf, go, gg]
                for j in range(8):
                    xwt = xwp.tile([NL, 512], BF16, tag="xwt")
                    nc.sync.dma_start(
                        out=xwt[:],
                        in_=d_xw[t * NL : (t + 1) * NL, j * 512 : (j + 1) * 512],
                    )
                    jc = slice(j * 512, (j + 1) * 512)
                    pp = packp.tile([128, 512], F32, tag="pack_ps")
                    for r in range(4):
                        for g in range(4):
                            kk = 4 * r + g
                            if kk < HC:
                                lhsT, rhs = hT_b[:, kk], wh[:, kk, jc]
                            else:
                                lhsT, rhs = aT_b[:, kk - HC], wa[:, kk - HC, jc]
                            nc.tensor.matmul(
                                pp[32 * g : 32 * g + 16, :], lhsT, rhs,
                                start=(r == 0), stop=(r == 3),
                                tile_position=(0, 32 * g),
                                skip_group_check=True,
                            )
                    pps = gat.tile([128, 512], BF16, tag="pps")
                    nc.vector.tensor_copy(pps[:], pp[:])
                    pj = actp.tile([NL, 512], F32, tag="act_ps")
                    nc.tensor.matmul(pj[:], ssum[:], pps[:], start=True, stop=False)
                    nc.tensor.matmul(
                        pj[:], id16b[:], xwt[:],
                        start=False, stop=True,
                    )
                    g = j // 2
                    half = (j % 2) * 512
                    func = (
                        mybir.ActivationFunctionType.Tanh
                        if g == 3
                        else mybir.ActivationFunctionType.Sigmoid
                    )
                    nc.scalar.activation(gates[g][:, half : half + 512], pj[:], func)

                # c = f*c + i*g ; h = o * tanh(c)
                fc = gat.tile([NL, H], F32, tag="fc")
                nc.vector.tensor_mul(fc[:], gf[:], c_sb[:])
                ig = gat.tile([NL, H], F32, tag="ig")
                nc.vector.tensor_mul(ig[:], gi[:], gg[:])
                nc.vector.tensor_add(c_sb[:], fc[:], ig[:])
                th = gat.tile([NL, H], F32, tag="th")
                nc.scalar.activation(th[:], c_sb[:], mybir.ActivationFunctionType.Tanh)
                nc.vector.tensor_mul(h_sb[:], go[:], th[:])

                nc.sync.dma_start(out=d_y[:, t, :], in_=h_sb[:])

                if t < T - 1:
                    transposes(h_sb)
                    attention()

    nc.compile()
    _cache["nc"] = nc
    return nc


def kernel(x, A, Wx, Wh, Wattn, b):
    x = np.ascontiguousarray(np.asarray(x, dtype=np.float32))
    A = np.ascontiguousarray(np.asarray(A, dtype=np.float32))
    Wxb = np.ascontiguousarray(np.asarray(Wx, dtype=np.float32).astype(np.float16))
    Whb = np.ascontiguousarray(np.asarray(Wh, dtype=np.float32).astype(np.float16))
    Wab = np.ascontiguousarray(np.asarray(Wattn, dtype=np.float32).astype(np.float16))
    b2 = np.ascontiguousarray(
        np.asarray(b, dtype=np.float32).reshape(1, K4).astype(np.float16)
    )

    ident = np.eye(16, dtype=np.float32)
    mask_mn = np.zeros((16, M * NL), dtype=np.float32)
    mask_nm = np.zeros((16, NL * M), dtype=np.float32)
    for a in range(M):
        for n in range(NL):
            mask_mn[n, a * NL + n] = 1.0  # [n, (m, n')]
            mask_nm[n, n * M + a] = 1.0   # [n', (n, m)]
    ones16 = np.ones((16, 128), dtype=np.float16)
    ssum = np.zeros((128, 16), dtype=np.float16)
    for g in range(4):
        for i in range(16):
            ssum[32 * g + i, i] = 1.0

    nc = _build()
    in_maps = []
    for k in range(NCORES):
        xs = x[k * NL : (k + 1) * NL]                     # [16, 64, 1024]
        xT = np.ascontiguousarray(
            xs.transpose(1, 0, 2).reshape(T * NL, D).T.astype(np.float16)
        )
        Ak = np.ascontiguousarray(A[k * NL : (k + 1) * NL].reshape(NL, H, M))
        in_maps.append(
            {
                "xT": xT,
                "A": Ak,
                "Wx": Wxb,
                "Wh": Whb,
                "Wa": Wab,
                "b": b2,
                "ident": ident,
                "mask_mn": mask_mn,
                "mask_nm": mask_nm,
                "ones16": ones16,
                "ssum": ssum,
            }
        )

    _cache["in_maps"] = in_maps
    res = run_bass_kernel_spmd(nc, in_maps, core_ids=list(range(NCORES)))
    out = np.concatenate([res.results[k]["y"] for k in range(NCORES)], axis=0)
    return out.astype(np.float32)


# revision 8
# speedup vs baseline: 1336.3563x; 1306.3600x over previous
"""Attention-LSTM (CaptioningRNN) Trainium2 kernel.

Strategy: data-parallel over the batch N=128 across 8 NeuronCores (16
samples/core), zero cross-core communication.  Per core:

  Phase 1:  xW = x_local @ Wx + b  (f32r matmuls, full 128-row PE tiles)
            -> bf16 DRAM scratch, rows ordered (t, n);
            c0 = h0 = mean_m(A_local)   (DVE reduce).
  Phase 2:  64 sequential LSTM steps.  The two recurrent GEMMs
            (h @ Wh, attn @ Wattn) run as bf16 matmuls: transposed state
            chunks are the stationary operand, SBUF-resident bf16 weights
            are the moving operand; xW_t is folded into the same PSUM
            accumulation via an identity matmul.  Attention:
              scores  = hT.T @ A_chunks (bf16 PE) -> diagonal extraction
                        via mask-multiply + reduce (DVE);
              softmax = ACT exp + DVE reduce/reciprocal;
              attn^T  = softmax weights broadcast across partitions with
                        a ones-matmul, then DVE multiply + reduce,
                        produced directly in the transposed layout the
                        next GEMM needs.
"""

import sys

sys.path.insert(0, "/opt/trn_rl_repo")

import ml_dtypes
import numpy as np

import concourse.bass as bass  # noqa: F401
import concourse.mybir as mybir
import concourse.tile as tile
from concourse import bacc
from concourse.bass_utils import run_bass_kernel_spmd

F32 = mybir.dt.float32
F32R = mybir.dt.float32r
BF16 = mybir.dt.float16  # IEEE fp16: same PE rate as bf16, 4x the mantissa precision

N, T, D, H = 128, 64, 1024, 1024
K4 = 4 * H            # 4096
NCORES = 8
NL = N // NCORES      # 16 samples per core
M = 16                # spatial positions (4x4)
HC = H // 128         # 8 h-chunks
SCALE = 1.0 / float(np.sqrt(H))

_cache = {}


def _build(steps=T):
    key = ("nc", steps)
    if key in _cache:
        return _cache[key]

    nc = bacc.Bacc("TRN2", target_bir_lowering=False)

    # ---- kernel I/O ----------------------------------------------------
    # xT rows are D, columns are (t, n) so step t's block is contiguous.
    d_xT = nc.dram_tensor("xT", [D, T * NL], BF16, kind="ExternalInput")
    d_A = nc.dram_tensor("A", [NL, H, M], F32, kind="ExternalInput")
    d_Wx = nc.dram_tensor("Wx", [D, K4], BF16, kind="ExternalInput")
    d_Wh = nc.dram_tensor("Wh", [H, K4], BF16, kind="ExternalInput")
    d_Wa = nc.dram_tensor("Wa", [H, K4], BF16, kind="ExternalInput")
    d_b = nc.dram_tensor("b", [1, K4], BF16, kind="ExternalInput")
    d_id = nc.dram_tensor("ident", [16, 16], F32, kind="ExternalInput")
    d_mmn = nc.dram_tensor("mask_mn", [16, 16 * 16], F32, kind="ExternalInput")
    d_mnm = nc.dram_tensor("mask_nm", [16, 16 * 16], F32, kind="ExternalInput")
    d_ones = nc.dram_tensor("ones16", [16, 128], BF16, kind="ExternalInput")
    d_ssum = nc.dram_tensor("ssum", [128, 16], BF16, kind="ExternalInput")
    d_y = nc.dram_tensor("y", [NL, T, H], F32, kind="ExternalOutput")

    d_xw = nc.dram_tensor("xw_scratch", [T * NL, K4], BF16)

    with tile.TileContext(nc) as tc:
      with tc.tile_pool(name="state", bufs=1) as stp:
        c_sb = stp.tile([NL, H], F32, tag="c")

        # ============== Phase 1: xW = x @ Wx + b, c0 ====================
        with (
            tc.tile_pool(name="p1w", bufs=1) as p1w,
            tc.tile_pool(name="p1s", bufs=2) as p1s,
            tc.tile_pool(name="p1p", bufs=4, space="PSUM") as p1p,
        ):
            wx = p1w.tile([128, HC, K4], BF16, tag="wx")
            nc.sync.dma_start(
                out=wx[:], in_=d_Wx.rearrange("(kc p) f -> p kc f", p=128)
            )
            bias = p1w.tile([1, K4], BF16, tag="bias")
            nc.sync.dma_start(out=bias[:], in_=d_b[:])
            ones1 = p1w.tile([1, 128], BF16, tag="ones1")
            nc.vector.memset(ones1[:], 1.0)

            xt = p1w.tile([128, HC, T * NL], BF16, tag="xt")
            nc.sync.dma_start(
                out=xt[:], in_=d_xT.rearrange("(kc p) r -> p kc r", p=128)
            )

            # c0 = mean over m of A  (layout [n, h])
            for hh in range(4):
                hs = H // 4
                a_n = p1s.tile([NL, hs, M], F32, tag="a_n")
                nc.sync.dma_start(
                    out=a_n[:], in_=d_A[:, hh * hs : (hh + 1) * hs, :]
                )
                csum = p1s.tile([NL, hs], F32, tag="csum")
                nc.vector.tensor_reduce(
                    csum[:], a_n[:], axis=mybir.AxisListType.X, op=mybir.AluOpType.add
                )
                nc.scalar.mul(c_sb[:, hh * hs : (hh + 1) * hs], csum[:], 1.0 / M)

            # xW GEMM over 8 row-chunks of (t, n)
            for mc in range(HC):
                for j in range(8):
                    pj = p1p.tile([128, 512], F32, tag="p1psum")
                    for kc in range(HC):
                        nc.tensor.matmul(
                            pj[:],
                            xt[:, kc, mc * 128 : (mc + 1) * 128],
                            wx[:, kc, j * 512 : (j + 1) * 512],
                            start=(kc == 0),
                            stop=False,
                        )
                    nc.tensor.matmul(
                        pj[:],
                        ones1[:],
                        bias[:, j * 512 : (j + 1) * 512],
                        start=False,
                        stop=True,
                    )
                    ob = p1s.tile([128, 512], BF16, tag="p1out")
                    nc.vector.tensor_copy(ob[:], pj[:])
                    nc.sync.dma_start(
                        out=d_xw[mc * 128 : (mc + 1) * 128, j * 512 : (j + 1) * 512],
                        in_=ob[:],
                    )

        # ============== Phase 2: recurrent loop =========================
        with (
            tc.tile_pool(name="wts", bufs=1) as wts,
            tc.tile_pool(name="stt", bufs=1) as stt,
            tc.tile_pool(name="xwp", bufs=2) as xwp,
            tc.tile_pool(name="gat", bufs=1) as gat,
            tc.tile_pool(name="sml", bufs=2) as sml,
            tc.tile_pool(name="big", bufs=1) as big,
            tc.tile_pool(name="actp", bufs=2, space="PSUM") as actp,
            tc.tile_pool(name="packp", bufs=2, space="PSUM") as packp,
            tc.tile_pool(name="trp", bufs=2, space="PSUM") as trp,
            tc.tile_pool(name="scp", bufs=1, space="PSUM") as scp,
        ):
            wh = wts.tile([128, HC, K4], BF16, tag="wh")
            nc.sync.dma_start(out=wh[:], in_=d_Wh.rearrange("(kc p) f -> p kc f", p=128))
            wa = wts.tile([128, HC, K4], BF16, tag="wa")
            nc.sync.dma_start(out=wa[:], in_=d_Wa.rearrange("(kc p) f -> p kc f", p=128))
            # A[n, hc*128+p, m] in transposed per-chunk layout, bf16
            at4 = wts.tile([128, HC, NL, M], BF16, tag="at4")
            for hc in range(HC):
                a_stage = sml.tile([128, NL, M], F32, tag="a_stage")
                nc.sync.dma_start(
                    out=a_stage[:],
                    in_=d_A.rearrange("n (hc p) m -> hc p n m", hc=HC)[hc],
                )
                nc.vector.tensor_copy(at4[:, hc], a_stage[:])
            id16 = wts.tile([16, 16], F32, tag="id16")
            nc.sync.dma_start(out=id16[:], in_=d_id[:])
            mask_mn = wts.tile([16, M, NL], F32, tag="mask_mn")
            nc.sync.dma_start(
                out=mask_mn[:], in_=d_mmn.rearrange("p (a c) -> p a c", a=M)
            )
            mask_nm = wts.tile([16, NL, M], F32, tag="mask_nm")
            nc.sync.dma_start(
                out=mask_nm[:], in_=d_mnm.rearrange("p (a c) -> p a c", a=NL)
            )
            ones16 = wts.tile([16, 128], BF16, tag="ones16")
            nc.sync.dma_start(out=ones16[:], in_=d_ones[:])
            id16b = wts.tile([16, 16], BF16, tag="id16b")
            nc.vector.tensor_copy(id16b[:], id16[:])
            ssum = wts.tile([128, 16], BF16, tag="ssum")
            nc.sync.dma_start(out=ssum[:], in_=d_ssum[:])

            h_sb = stt.tile([NL, H], F32, tag="h")
            nc.vector.tensor_copy(h_sb[:], c_sb[:])

            hT_b = stt.tile([128, HC, NL], BF16, tag="hT_b")
            aT_b = stt.tile([128, HC, NL], BF16, tag="aT_b")

            def transposes(src_sb):
                for k in range(HC):
                    pt = trp.tile([128, NL], F32, tag="trps")
                    nc.tensor.transpose(
                        pt[:], src_sb[:, k * 128 : (k + 1) * 128], id16[:]
                    )
                    nc.vector.tensor_copy(hT_b[:, k], pt[:])

            def attention():
                # scores: S[n, (m, n')] accumulated over h-chunks (bf16 PE)
                ps = scp.tile([16, M * NL], F32, tag="sc_ps")
                for k in range(HC):
                    nc.tensor.matmul(
                        ps[:],
                        hT_b[:, k],
                        at4[:, k].rearrange("p n m -> p m n"),
                        start=(k == 0),
                        stop=(k == HC - 1),
                    )
                smul = sml.tile([16, M, NL], F32, tag="smul")
                nc.vector.tensor_mul(
                    smul[:], ps[:].rearrange("p (m n) -> p m n", m=M), mask_mn[:]
                )
                sc = sml.tile([16, M], F32, tag="sc")
                nc.vector.tensor_reduce(
                    sc[:], smul[:], axis=mybir.AxisListType.X, op=mybir.AluOpType.add
                )
                # softmax (1/sqrt(H) scale folded into exp)
                mx = sml.tile([16, 1], F32, tag="mx")
                nc.vector.tensor_reduce(
                    mx[:], sc[:], axis=mybir.AxisListType.X, op=mybir.AluOpType.max
                )
                nb = sml.tile([16, 1], F32, tag="nb")
                nc.scalar.mul(nb[:], mx[:], -SCALE)
                ex = sml.tile([16, M], F32, tag="ex")
                nc.scalar.activation(
                    ex[:], sc[:], mybir.ActivationFunctionType.Exp,
                    bias=nb[:], scale=SCALE,
                )
                sm = sml.tile([16, 1], F32, tag="sm")
                nc.vector.tensor_reduce(
                    sm[:], ex[:], axis=mybir.AxisListType.X, op=mybir.AluOpType.add
                )
                rc = sml.tile([16, 1], F32, tag="rc")
                nc.vector.reciprocal(rc[:], sm[:])
                w16 = sml.tile([16, M], F32, tag="w16")
                nc.vector.tensor_scalar_mul(w16[:], ex[:], rc[:])
                # wB[p, (n, m)] = w[n, m] on every partition p
                wd = sml.tile([16, NL, M], BF16, tag="wd")
                nc.vector.tensor_mul(
                    wd[:],
                    w16[:].unsqueeze(1).broadcast_to([16, NL, M]),
                    mask_nm[:],
                )
                pwb = scp.tile([128, NL * M], F32, tag="wb_ps")
                nc.tensor.matmul(
                    pwb[:],
                    ones16[:],
                    wd[:].rearrange("p n m -> p (n m)"),
                    start=True,
                    stop=True,
                )
                wbs = sml.tile([128, NL, M], BF16, tag="wbs")
                nc.vector.tensor_copy(
                    wbs[:], pwb[:].rearrange("p (n m) -> p n m", n=NL)
                )
                # attnT[p, (hc, n)] = sum_m A[n, hc*128+p, m] * w[n, m]
                tmp = big.tile([128, HC, NL, M], BF16, tag="attmp")
                nc.vector.tensor_mul(
                    tmp[:],
                    at4[:],
                    wbs[:].unsqueeze(1).broadcast_to([128, HC, NL, M]),
                )
                atf = sml.tile([128, HC, NL], F32, tag="atf")
                nc.vector.tensor_reduce(
                    atf[:], tmp[:], axis=mybir.AxisListType.X, op=mybir.AluOpType.add
                )
                nc.vector.tensor_copy(aT_b[:], atf[:])

            # initial state: h = c0; hT and attn for step 0
            transposes(h_sb)
            attention()

            for t in range(steps):
                gi = gat.tile([NL, H], F32, tag="gi")
                gf = gat.tile([NL, H], F32, tag="gf")
                go = gat.tile([NL, H], F32, tag="go")
                gg = gat.tile([NL, H], F32, tag="gg")
                gates = [gi, gf, go, gg]
                for j in range(8):
                    xwt = xwp.tile([NL, 512], BF16, tag="xwt")
                    nc.sync.dma_start(
                        out=xwt[:],
                        in_=d_xw[t * NL : (t + 1) * NL, j * 512 : (j + 1) * 512],
                    )
                    jc = slice(j * 512, (j + 1) * 512)
                    pp = packp.tile([128, 512], F32, tag="pack_ps")
                    for r in range(4):
                        for g in range(4):
                            kk = 4 * r + g
                            if kk < HC:
                                lhsT, rhs = hT_b[:, kk], wh[:, kk, jc]
                            else:
                                lhsT, rhs = aT_b[:, kk - HC], wa[:, kk - HC, jc]
                            nc.tensor.matmul(
                                pp[32 * g : 32 * g + 16, :], lhsT, rhs,
                                start=(r == 0), stop=(r == 3),
                                tile_position=(0, 32 * g),
                                skip_group_check=True,
                            )
                    pps = gat.tile([128, 512], BF16, tag="pps")
                    nc.vector.tensor_copy(pps[:], pp[:])
                    pj = actp.tile([NL, 512], F32, tag="act_ps")
                    nc.tensor.matmul(pj[:], ssum[:], pps[:], start=True, stop=False)
                    nc.tensor.matmul(
                        pj[:], id16b[:], xwt[:],
                        start=False, stop=True,
                    )
                    g = j // 2
                    half = (j % 2) * 512
                    func = (
                        mybir.ActivationFunctionType.Tanh
                        if g == 3
                        else mybir.ActivationFunctionType.Sigmoid
                    )
                    nc.scalar.activation(gates[g][:, half : half + 512], pj[:], func)

                # c = f*c + i*g ; h = o * tanh(c)
                fc = gat.tile([NL, H], F32, tag="fc")
                nc.vector.tensor_mul(fc[:], gf[:], c_sb[:])
                ig = gat.tile([NL, H], F32, tag="ig")
                nc.vector.tensor_mul(ig[:], gi[:], gg[:])
                nc.vector.tensor_add(c_sb[:], fc[:], ig[:])
                th = gat.tile([NL, H], F32, tag="th")
                nc.scalar.activation(th[:], c_sb[:], mybir.ActivationFunctionType.Tanh)
                nc.vector.tensor_mul(h_sb[:], go[:], th[:])

                nc.sync.dma_start(out=d_y[:, t, :], in_=h_sb[:])

                if t < steps - 1:
                    transposes(h_sb)
                    attention()

    nc.compile()
    _cache[key] = nc
    return nc


def _prepare(x, A, Wx, Wh, Wattn, b):
    x = np.ascontiguousarray(np.asarray(x, dtype=np.float32))
    A = np.ascontiguousarray(np.asarray(A, dtype=np.float32))
    Wxb = np.ascontiguousarray(np.asarray(Wx, dtype=np.float32).astype(np.float16))
    Whb = np.ascontiguousarray(np.asarray(Wh, dtype=np.float32).astype(np.float16))
    Wab = np.ascontiguousarray(np.asarray(Wattn, dtype=np.float32).astype(np.float16))
    b2 = np.ascontiguousarray(
        np.asarray(b, dtype=np.float32).reshape(1, K4).astype(np.float16)
    )

    ident = np.eye(16, dtype=np.float32)
    mask_mn = np.zeros((16, M * NL), dtype=np.float32)
    mask_nm = np.zeros((16, NL * M), dtype=np.float32)
    for a in range(M):
        for n in range(NL):
            mask_mn[n, a * NL + n] = 1.0  # [n, (m, n')]
            mask_nm[n, n * M + a] = 1.0   # [n', (n, m)]
    ones16 = np.ones((16, 128), dtype=np.float16)
    ssum = np.zeros((128, 16), dtype=np.float16)
    for g in range(4):
        for i in range(16):
            ssum[32 * g + i, i] = 1.0

    in_maps = []
    for k in range(NCORES):
        xs = x[k * NL : (k + 1) * NL]                     # [16, 64, 1024]
        xT = np.ascontiguousarray(
            xs.transpose(1, 0, 2).reshape(T * NL, D).T.astype(np.float16)
        )
        Ak = np.ascontiguousarray(A[k * NL : (k + 1) * NL].reshape(NL, H, M))
        in_maps.append(
            {
                "xT": xT,
                "A": Ak,
                "Wx": Wxb,
                "Wh": Whb,
                "Wa": Wab,
                "b": b2,
                "ident": ident,
                "mask_mn": mask_mn,
                "mask_nm": mask_nm,
                "ones16": ones16,
                "ssum": ssum,
            }
        )

    _cache["in_maps"] = in_maps
    return in_maps


def kernel(x, A, Wx, Wh, Wattn, b):
    nc = _build()
    in_maps = _prepare(x, A, Wx, Wh, Wattn, b)
    res = run_bass_kernel_spmd(nc, in_maps, core_ids=list(range(NCORES)))
    out = np.concatenate([res.results[k]["y"] for k in range(NCORES)], axis=0)
    return out.astype(np.float32)
